# revision 8
# baseline (speedup 1.0000x reference)
import math

import jax
import jax.numpy as jnp
import numpy as np

# ANI-style AEVComputer, data-parallel over the molecule axis N=8:
# one molecule per NeuronCore (AEVs are per-atom; no cross-device comm).
# All on-device tensors are kept rank<=2 (the neuron compiler's tiling pass
# asserts on rank-3 same-size-axes compute DAGs); pair/triplet broadcasts are
# expressed as static-index gathers and species contractions as 2D matmuls.
NUM_SPECIES = 4
RCR = 5.2  # radial cutoff (Angstrom)
RCA = 3.5  # angular cutoff (Angstrom)
N, A = 8, 64
N_CORES = 8


def _triu_index_np(num_species):
    s1, s2 = np.triu_indices(num_species, 0)
    pair = np.arange(len(s1))
    ret = np.zeros((num_species, num_species), dtype=np.int32)
    ret[s1, s2] = pair
    ret[s2, s1] = pair
    return ret


def _int_pow(x, n):
    # exact exponentiation-by-squaring for integer n (Zeta=32 in practice)
    result = None
    p = x
    while n > 0:
        if n & 1:
            result = p if result is None else result * p
        n >>= 1
        if n:
            p = p * p
    return result


def _make_aev_fn(EtaR, ShfR, EtaA, Zeta, ShfA, ShfZ):
    """Bake the (tiny) parameter grids in as compile-time constants."""
    etar = float(np.asarray(EtaR).ravel()[0])
    shfr = np.asarray(ShfR, np.float64)  # (R,)
    etaa = float(np.asarray(EtaA).ravel()[0])
    zeta = float(np.asarray(Zeta).ravel()[0])
    shfa = np.asarray(ShfA, np.float64)  # (SA,)
    cos_sz = np.cos(np.asarray(ShfZ, np.float64))  # (SZ,)
    sin_sz = np.sin(np.asarray(ShfZ, np.float64))
    n_r = shfr.shape[0]
    n_a = shfa.shape[0]
    n_z = cos_sz.shape[0]
    S = NUM_SPECIES
    P = S * (S + 1) // 2
    zeta_is_int = float(zeta).is_integer() and zeta > 0

    # static index vectors for the (j,k) -> jk flattening
    jj = np.repeat(np.arange(A), A)  # (A*A,)
    kk = np.tile(np.arange(A), A)  # (A*A,)
    noteye_flat = (jj != kk).astype(np.float32).reshape(1, A * A)

    def aev_one(species, coordinates):
        barrier = jax.lax.optimization_barrier
        coords_b = barrier(coordinates)
        gram = coordinates @ coords_b.T  # (A,A)
        gd = jnp.sum(jnp.square(coordinates), axis=-1)  # (A,)
        gd_b = barrier(gd)
        d2 = jnp.maximum(gd[:, None] + gd_b[None, :] - 2.0 * gram, 0.0)
        d2_off = d2 + jnp.asarray(np.eye(A, dtype=np.float32))  # diag -> 1.0
        dist = jnp.sqrt(d2_off)  # (A,A); safe sqrt on diagonal
        offdiag = jnp.asarray(1.0 - np.eye(A, dtype=np.float32))

        sp1h = jax.nn.one_hot(species, S, dtype=jnp.float32)  # (A,S)

        # ---------- radial sub-AEV (features emitted in (r, s) order) ----
        rmask = jnp.where(dist <= RCR, 1.0, 0.0) * offdiag
        fc_r = 0.5 * jnp.cos(dist * (math.pi / RCR)) + 0.5
        pre_r = 0.25 * fc_r * rmask  # (A,A)
        rad_cols = []
        for r in range(n_r):
            radr = jnp.exp(-etar * jnp.square(dist - float(shfr[r]))) * pre_r
            rad_cols.append(radr @ sp1h)  # (A,S)
        radial_dev = jnp.concatenate(rad_cols, axis=1)  # (A, R*S) order (r,s)

        # ---------- angular sub-AEV ----------
        amask = jnp.where(dist <= RCA, 1.0, 0.0) * offdiag
        fc_a = 0.5 * jnp.cos(dist * (math.pi / RCA)) + 0.5
        mfc = fc_a * amask  # (A,A)
        inv_d = amask / dist  # masked inverse distance (diag dist=1 -> 0)

        gram_b = barrier(gram)
        gram_f = gram.reshape(1, A * A)
        gJ = jnp.take(gram, jj, axis=1)  # (A, A*A): G[i, j(x)]
        gK = jnp.take(gram_b, kk, axis=1)  # (A, A*A): G[i, k(x)]
        dotv = gram_f - gJ - gK + gd[:, None]  # diff_ij . diff_ik

        inv_d_b = barrier(inv_d)
        invJ = jnp.take(inv_d, jj, axis=1)
        invK = jnp.take(inv_d_b, kk, axis=1)
        cosang = 0.95 * dotv * (invJ * invK)  # (A, A*A)
        sinang = jnp.sqrt(jnp.maximum(1.0 - jnp.square(cosang), 0.0))

        dist_b = barrier(dist)
        dJ = jnp.take(dist, jj, axis=1)
        dK = jnp.take(dist_b, kk, axis=1)
        dsum_h = 0.5 * (dJ + dK)  # (A, A*A)

        mfc_b = barrier(mfc)
        mfcJ = jnp.take(mfc, jj, axis=1)
        mfcK = jnp.take(mfc_b, kk, axis=1)
        wmask = mfcJ * mfcK * jnp.asarray(noteye_flat)  # (A, A*A)

        # species-pair one-hot over the flattened (j,k) axis: (A*A, P)
        triu = jnp.asarray(_triu_index_np(S))
        pair_idx = triu[species[jj], species[kk]]  # (A*A,)
        pair1h = jax.nn.one_hot(pair_idx, P, dtype=jnp.float32)  # (A*A, P)
        pair1h = barrier(pair1h)

        f1s = []
        for z in range(n_z):
            base = 0.5 * (1.0 + cosang * float(cos_sz[z]) + sinang * float(sin_sz[z]))
            f1s.append(_int_pow(base, int(zeta)) if zeta_is_int else base**zeta)

        ang_cols = []
        for a in range(n_a):
            f2a = jnp.exp(-etaa * jnp.square(dsum_h - float(shfa[a])))
            w2a = f2a * wmask  # (A, A*A)
            for z in range(n_z):
                ga = w2a * f1s[z]
                ang_cols.append(ga @ pair1h)  # (A, P)
        # (A, SA*SZ*P), order (a, z, p); includes the 2.0 * 0.5 = 1.0 factor
        angular_dev = jnp.concatenate(ang_cols, axis=1)

        return jnp.concatenate([radial_dev, angular_dev], axis=1)

    n_feat = n_r * S + n_a * n_z * P

    # host-side column permutation: target -> source
    perm = np.zeros(n_feat, dtype=np.int64)
    for s in range(S):
        for r in range(n_r):
            perm[s * n_r + r] = r * S + s
    base_t = S * n_r
    for p in range(P):
        for a in range(n_a):
            for z in range(n_z):
                perm[base_t + p * (n_a * n_z) + a * n_z + z] = (
                    base_t + (a * n_z + z) * P + p
                )

    return aev_one, perm


_cache = {}


def _get_pmapped(params_key, *params):
    if params_key not in _cache:
        fn, perm = _make_aev_fn(*params)
        pm = jax.pmap(fn, in_axes=(0, 0), devices=jax.devices()[:N_CORES])
        _cache[params_key] = (pm, perm)
    return _cache[params_key]


def kernel(species, coordinates, EtaR, ShfR, EtaA, Zeta, ShfA, ShfZ):
    species_in = species
    sp = jnp.asarray(np.asarray(species), dtype=jnp.int32)
    xyz = jnp.asarray(np.asarray(coordinates), dtype=jnp.float32)
    params = [np.asarray(p, np.float32) for p in (EtaR, ShfR, EtaA, Zeta, ShfA, ShfZ)]
    key = tuple(p.tobytes() for p in params)
    f, perm = _get_pmapped(key, *params)
    out = np.asarray(f(sp, xyz), dtype=np.float32)  # (N, A, n_feat) device order
    aevs = out[:, :, perm]
    return (species_in, aevs)
